# revision 19
# baseline (speedup 1.0000x reference)
"""GCN message-passing kernel for Trainium2, 8 NeuronCores.

Model (see reference):
    h   = relu(GCNConv(x, edge_index; W1, b1))      # [N, 128]
    p   = segment_max(h, batch, 128 graphs)          # [128, 128]
    out = log_softmax(p @ W2 + b2)                   # [128, 2]

GCNConv with self loops and symmetric norm decomposes as
    out = D^-1/2 * A * D^-1/2 * (x @ W1) + b1
so we compute H' = dinv * (x@W1) row-scaled, aggregate H'[src] into dst with
an unweighted segment-sum, then row-scale by dinv[dst] again.

Distribution (8 cores):
  * nodes row-sharded contiguously: core c owns nodes [c*6250, (c+1)*6250)
  * core c computes its H' slice; two chunked AllGathers (first 28 tiles,
    then the last 21) so the second collective overlaps the first gathers.
  * edges sharded by dst ownership (sorted by dst on host); each core
    aggregates its own dst rows:  for each 128-dst-node tile, gather
    H'[src] rows via indirect DMA, build a one-hot selection matrix from
    the dst indices (iota == dstrel), and matmul-accumulate into PSUM.
    Edges are banded by 64-dst group within the tile so the one-hot is
    [slots, 64] per chunk (half the DVE area; DVE 2-port ops lock the Q7
    descriptor generator out of SBUF, so DVE time is Pool stall time).
    Self-loop edges are NOT gathered: the local H' tile is added with
    identity matmuls per 64-band instead.
  * graph boundaries align with the node shard (6250 nodes = exactly 16
    graphs per core), so segment_max + head are fully core-local.
  * final [16,2] per-core outputs are concatenated on host.
"""

import math

import numpy as np

N_NODES = 50000
N_EDGES = 1600000
N_FEAT = 512
N_HID = 128
N_CLASSES = 2
NUM_GRAPHS = 128
NCORES = 8
NPC = N_NODES // NCORES          # 6250 nodes per core
GPC = NUM_GRAPHS // NCORES       # 16 graphs per core
P = 128
NT = (NPC + P - 1) // P          # 49 dst tiles per core (last tile 106 valid)
KF = N_FEAT // P                 # 4 K-chunks for x @ W1
GT = 7                           # tiles per phase-A DMA group (7*7 = 49)
NTA = 28                         # tiles in collective piece A (4 groups)
NTB = NT - NTA                   # 21 tiles in piece B (3 groups)
RA = NTA * P                     # 3584 local rows in piece A
RB = NTB * P                     # 2688 local rows in piece B
W = 64                           # dst band width (2 bands per tile)
NB = P // W                      # 2 bands

_PROGRAM_CACHE: dict = {}
LAST_RESULTS = None              # BassKernelResults of the most recent run


def _host_prep(x, W1, b1, W2, b2, edge_index, batch):
    """All integer/index preprocessing + input shard construction."""
    x = np.asarray(x, dtype=np.float32)
    W1 = np.asarray(W1, dtype=np.float32)
    b1 = np.asarray(b1, dtype=np.float32)
    W2 = np.asarray(W2, dtype=np.float32)
    b2 = np.asarray(b2, dtype=np.float32)
    ei = np.asarray(edge_index)
    batch = np.asarray(batch)

    src = ei[0].astype(np.int32)
    dst = ei[1].astype(np.int32)

    # in-degree (with self loops); float input to the device rsqrt.
    # Self loops are handled by identity matmuls on the local H' tile,
    # so they are excluded from the gathered edge list (but kept in deg).
    deg = (np.bincount(dst, minlength=N_NODES) + 1).astype(np.float32)

    # gather table row: piece A = local rows [0, RA) of every core packed
    # first (rows c*RA + l), piece B = local rows [RA, NPC) (rows
    # 8*RA + c*RB + (l-RA)).  "half" h is the piece id; both pieces'
    # row spaces stay below 32768 so int16 gather indices reach them.
    lcl = src % NPC
    half_of = (lcl >= RA).astype(np.int64)
    row_of = np.where(half_of == 0,
                      (src // NPC) * RA + lcl,
                      (src // NPC) * RB + (lcl - RA))
    tid = (dst // NPC) * NT + (dst % NPC) // P               # global dst tile
    # sort by (dst tile, table piece, src row): the src-row order makes the
    # SDMA m2s reads walk monotonically increasing HBM addresses (row-buffer
    # locality); slot order within a tile is free (one-hot maps slot->dst).
    order = np.lexsort((row_of, half_of, tid))
    row_s = row_of[order]
    dst_s = dst[order]
    key_s = tid[order] * 2 + half_of[order]

    # graph boundaries from the actual batch tensor; must align to the shard
    gbs = np.searchsorted(batch, np.arange(NUM_GRAPHS + 1))
    gb_local = gbs[:GPC + 1].astype(np.int64).copy()
    for c in range(NCORES):
        seg = gbs[c * GPC:(c + 1) * GPC + 1] - c * NPC
        assert np.array_equal(seg, gb_local), "graph/node shard misalignment"

    # per (core, tile, half) edge counts -> common chunk schedule
    # (tid encodes the core, so key_s separates cores already)
    cell_ofs = np.searchsorted(key_s, np.arange(NCORES * NT * 2 + 1))
    cnt = (cell_ofs[1:] - cell_ofs[:-1]).reshape(NCORES, NT, 2)
    chunks = -(-cnt.max(axis=0) // P)                        # [NT, 2]
    q0 = np.zeros((NT, 2), dtype=np.int64)
    ct0 = np.zeros((NT, 2), dtype=np.int64)
    acc_c = acc_q = 0
    for t in range(NT):
        for h in range(2):
            q0[t, h] = acc_q
            ct0[t, h] = acc_c
            acc_c += chunks[t, h]
            acc_q += int(chunks[t, h]) * 8                   # 128/16 cols16
    ctot = int(acc_c)
    qtot = int(acc_q)

    gidx = np.zeros((NCORES, P, qtot), dtype=np.int16)
    dstr = np.full((NCORES, P, ctot), 255.0, dtype=np.float16)
    for c in range(NCORES):
        for t in range(NT):
            for h in range(2):
                i = (c * NT + t) * 2 + h
                e0 = cell_ofs[i]
                n = int(cnt[c, t, h])
                K = int(chunks[t, h]) * P                    # padded len
                if K == 0:
                    continue
                idx = np.zeros(K, dtype=np.int16)            # pad: row 0
                idx[:n] = row_s[e0:e0 + n].astype(np.int16)
                gidx[c, :, q0[t, h]:q0[t, h] + K // 16] = \
                    np.tile(idx.reshape(K // 16, 16).T, (8, 1))
                if n:
                    s = np.arange(n)
                    dstr[c, s % P, ct0[t, h] + s // P] = \
                        (dst_s[e0:e0 + n]
                         - (c * NPC + t * P)).astype(np.float16)

    # x transposed + padded to the tile grid; deg per-core in [128, NT] layout
    xT = np.ascontiguousarray(x.T).astype(np.float16)       # [512, 50000]
    xT_pad = np.zeros((NCORES, N_FEAT, NT * P), dtype=np.float16)
    deg_cols = np.ones((NCORES, P, NT), dtype=np.float32)
    for c in range(NCORES):
        xT_pad[c, :, :NPC] = xT[:, c * NPC:(c + 1) * NPC]
        d = deg[c * NPC:(c + 1) * NPC]                      # [6250]
        dp = np.ones(NT * P, dtype=np.float32)
        dp[:NPC] = d
        deg_cols[c] = dp.reshape(NT, P).T

    iota_mat = np.tile(np.arange(P, dtype=np.float16), (P, 1))
    ident_f16 = np.eye(P, dtype=np.float16)
    b1_col = b1.reshape(N_HID, 1).astype(np.float32)
    b2_mat = np.tile(b2[None, :], (GPC, 1)).astype(np.float32)

    in_maps = []
    for c in range(NCORES):
        in_maps.append({
            "xT": xT_pad[c],
            "w1": W1.astype(np.float16),
            "b1c": b1_col,
            "w2": W2,
            "b2m": b2_mat,
            "degc": deg_cols[c],
            "gidx": gidx[c],
            "dstr": dstr[c],
            "iot": iota_mat,
            "idn": ident_f16,
        })
    chunks_key = tuple(tuple(int(v) for v in row) for row in chunks)
    return chunks_key, tuple(int(v) for v in gb_local), in_maps


def _build_program(chunks, gb_local):
    import concourse.bacc as bacc
    import concourse.bass as bass
    import concourse.mybir as mybir
    import concourse.tile as tile
    from concourse.masks import make_identity

    f32 = mybir.dt.float32
    f16 = mybir.dt.float16
    i16 = mybir.dt.int16
    # chunks: [NT][2]; offsets mirror _host_prep
    q0 = [[0, 0] for _ in range(NT)]
    ct0 = [[0, 0] for _ in range(NT)]
    acc_c = acc_q = 0
    for t in range(NT):
        for h in range(2):
            q0[t][h] = acc_q
            ct0[t][h] = acc_c
            acc_c += chunks[t][h]
            acc_q += chunks[t][h] * 8
    ctot = acc_c
    qtot = acc_q

    nc = bacc.Bacc("TRN2", target_bir_lowering=False, debug=False,
                   num_devices=NCORES, num_swdge_queues=4,
                   dynamic_dma_scratch_size=65536)

    xT = nc.dram_tensor("xT", [N_FEAT, NT * P], f16, kind="ExternalInput")
    w1 = nc.dram_tensor("w1", [N_FEAT, N_HID], f16, kind="ExternalInput")
    b1c = nc.dram_tensor("b1c", [N_HID, 1], f32, kind="ExternalInput")
    w2 = nc.dram_tensor("w2", [N_HID, N_CLASSES], f32, kind="ExternalInput")
    b2m = nc.dram_tensor("b2m", [GPC, N_CLASSES], f32, kind="ExternalInput")
    degc = nc.dram_tensor("degc", [P, NT], f32, kind="ExternalInput")
    gidx = nc.dram_tensor("gidx", [P, qtot], i16, kind="ExternalInput")
    dstr = nc.dram_tensor("dstr", [P, ctot], f16, kind="ExternalInput")
    iot = nc.dram_tensor("iot", [P, P], f16, kind="ExternalInput")
    idn = nc.dram_tensor("idn", [P, P], f16, kind="ExternalInput")
    out = nc.dram_tensor("out", [GPC, N_CLASSES], f32, kind="ExternalOutput")

    with tile.TileContext(nc) as tc:
        with (
            tc.tile_pool(name="const", bufs=1) as constp,
            tc.tile_pool(name="xg", bufs=2) as xgp,
            tc.tile_pool(name="oh", bufs=3) as ohp,
            tc.tile_pool(name="gg", bufs=6) as ggp,
            tc.tile_pool(name="hr", bufs=3) as hrp,
            tc.tile_pool(name="ps_h", bufs=2, space="PSUM") as psh,
            tc.tile_pool(name="ps_a", bufs=2, space="PSUM") as psa,
            tc.tile_pool(name="ps_t", bufs=2, space="PSUM") as pst,
            tc.tile_pool(name="dram", bufs=1, space="DRAM") as dramp,
        ):
            # ---- constants / persistent state ----
            w1_sb = constp.tile([P, N_FEAT], f16)            # [p, (k n)]
            nc.sync.dma_start(
                out=w1_sb[:].rearrange("p (k n) -> p k n", k=KF),
                in_=w1.ap().rearrange("(k p) n -> p k n", p=P))
            iota_sb = constp.tile([P, P], f16)
            nc.sync.dma_start(out=iota_sb[:], in_=iot.ap())
            idn_sb = constp.tile([P, P], f16)
            nc.sync.dma_start(out=idn_sb[:], in_=idn.ap())
            b1_sb = constp.tile([N_HID, 1], f32)
            nc.sync.dma_start(out=b1_sb[:], in_=b1c.ap())
            w2_sb = constp.tile([N_HID, N_CLASSES], f32)
            nc.sync.dma_start(out=w2_sb[:], in_=w2.ap())
            b2_sb = constp.tile([GPC, N_CLASSES], f32)
            nc.sync.dma_start(out=b2_sb[:], in_=b2m.ap())
            ident = constp.tile([P, P], f32)
            make_identity(nc, ident[:])

            deg_sb = constp.tile([P, NT], f32)
            nc.sync.dma_start(out=deg_sb[:], in_=degc.ap())
            rec_sb = constp.tile([P, NT], f32)
            nc.vector.reciprocal(rec_sb[:], deg_sb[:])
            dinv_sb = constp.tile([P, NT], f32)
            nc.scalar.activation(dinv_sb[:], rec_sb[:],
                                 mybir.ActivationFunctionType.Sqrt)

            gidx_sb = constp.tile([P, qtot], i16)
            nc.sync.dma_start(out=gidx_sb[:], in_=gidx.ap())
            dstr_sb = constp.tile([P, ctot], f16)
            nc.sync.dma_start(out=dstr_sb[:], in_=dstr.ap())

            hp_all = constp.tile([P, NT * P], f16)           # local H' tiles
            hT_sb = constp.tile([P, NT * P], f16)            # [hid, node]

            cc_in = dramp.tile([NT * P, N_HID], f16)
            cc_fa = dramp.tile([NCORES * RA, N_HID], f16, addr_space="Shared")
            cc_fb = dramp.tile([NCORES * RB, N_HID], f16, addr_space="Shared")

            # ---- phase A: H' = dinv * (x @ W1), per 128-node tile ----
            # xT loads and cc_in stores batched in groups of GT tiles.
            for g in range(NT // GT):
                xg_t = xgp.tile([P, KF * GT * P], f16, tag="xg")
                nc.sync.dma_start(
                    out=xg_t[:].rearrange("p (k c) -> p k c", k=KF),
                    in_=xT.ap()[:, g * GT * P:(g + 1) * GT * P]
                        .rearrange("(k p) c -> p k c", p=P))
                for ti in range(GT):
                    t = g * GT + ti
                    ph = psh.tile([P, N_HID], f32, tag="ph")
                    for k in range(KF):
                        nc.tensor.matmul(
                            ph[:],
                            lhsT=xg_t[:, (k * GT + ti) * P:(k * GT + ti + 1) * P],
                            rhs=w1_sb[:, k * P:(k + 1) * P],
                            start=(k == 0), stop=(k == KF - 1))
                    nc.scalar.activation(hp_all[:, t * P:(t + 1) * P], ph[:],
                                         mybir.ActivationFunctionType.Copy,
                                         scale=dinv_sb[:, t:t + 1])
                nc.sync.dma_start(
                    out=cc_in[g * GT * P:(g + 1) * GT * P, :]
                        .rearrange("(T p) m -> p T m", p=P),
                    in_=hp_all[:, g * GT * P:(g + 1) * GT * P]
                        .rearrange("p (T m) -> p T m", m=P))
                if g == NTA // GT - 1:
                    # piece A complete -> overlap its AllGather with the
                    # rest of phase A and start gathers under collective B
                    nc.gpsimd.collective_compute(
                        "AllGather", mybir.AluOpType.bypass,
                        replica_groups=[list(range(NCORES))],
                        ins=[cc_in[0:RA, :]],
                        outs=[cc_fa[:, :]],
                    )
            nc.gpsimd.collective_compute(
                "AllGather", mybir.AluOpType.bypass,
                replica_groups=[list(range(NCORES))],
                ins=[cc_in[RA:NT * P, :]],
                outs=[cc_fb[:, :]],
            )

            # ---- phase B: aggregate per dst tile ----
            # software-pipelined: piece-A gathers run LA tiles ahead of the
            # piece-B gathers + compute, so the early piece-A work hides
            # collective B's latency and keeps more DMA queues in flight.
            qn = 0
            LA = 4
            g_tiles = [None] * NT

            def emit_gathers(t, h, g_t, goff):
                # <=1024-idx sub-gathers: single_packet coalesces each into
                # one 64-desc packet per engine, which the SDMA drains
                # pipelined (~4x the per-descriptor rate of
                # descriptor-per-packet mode; >64-desc packets hang).
                nonlocal qn
                ch = chunks[t][h]
                coff = 0
                while coff < ch:
                    sub = min(8, ch - coff)
                    K = sub * P
                    qo = q0[t][h] + coff * 8
                    nc.gpsimd.dma_gather(
                        out_ap=g_t[:, (goff + coff) * P:
                                   (goff + coff + sub) * P]
                            .rearrange("p (j f) -> p j f", f=N_HID),
                        in_ap=(cc_fa if h == 0 else cc_fb)[:, :],
                        idxs_ap=gidx_sb[:, qo:qo + K // 16],
                        num_idxs=K,
                        num_idxs_reg=K,
                        elem_size=N_HID,
                        single_packet=True,
                        queue_num=qn % 4,
                    )
                    qn += 1
                    coff += sub

            for i in range(NT + LA):
                if i < NT:
                    cn_i = chunks[i][0] + chunks[i][1]
                    g_tiles[i] = ggp.tile([P, cn_i * P], f16, tag="g",
                                          name=f"g_t{i}")
                    emit_gathers(i, 0, g_tiles[i], 0)
                if i < LA:
                    continue
                t = i - LA
                cn = chunks[t][0] + chunks[t][1]
                g_t = g_tiles[t]
                g_tiles[t] = None
                emit_gathers(t, 1, g_t, chunks[t][0])
                oh_t = ohp.tile([P, cn * P], f16, tag="oh")
                # one-hot: oh[p, (c n)] = (iota[n] == dstrel[p, c])
                nc.vector.tensor_tensor(
                    out=oh_t[:].rearrange("p (c n) -> p c n", n=P),
                    in0=iota_sb[:].rearrange("p (one n) -> p one n", one=1)
                        .to_broadcast([P, cn, P]),
                    in1=dstr_sb[:, ct0[t][0]:ct0[t][0] + cn]
                        .rearrange("p (c one) -> p c one", one=1)
                        .to_broadcast([P, cn, P]),
                    op=mybir.AluOpType.is_equal,
                )
                pa = psa.tile([P, N_HID], f32, tag="pa")
                for j in range(cn):
                    nc.tensor.matmul(pa[:],
                                     lhsT=oh_t[:, j * P:(j + 1) * P],
                                     rhs=g_t[:, j * P:(j + 1) * P],
                                     start=(j == 0), stop=False)
                # self loops: += I^T @ H'_local  (H' rows for this tile)
                nc.tensor.matmul(pa[:], lhsT=idn_sb[:],
                                 rhs=hp_all[:, t * P:(t + 1) * P],
                                 start=False, stop=True)
                # epilogue on ACT + PE only (DVE stalls badly while the Q7
                # gather ucode is hammering SBUF): scale rows by dinv[dst],
                # transpose to [hid, node], then relu(x + b1) with b1 as the
                # per-partition ACT bias in the transposed layout.
                hr = hrp.tile([P, N_HID], f32, tag="hr")
                nc.scalar.activation(hr[:], pa[:],
                                     mybir.ActivationFunctionType.Copy,
                                     scale=dinv_sb[:, t:t + 1])
                ptr = pst.tile([P, P], f32, tag="pt")
                nc.tensor.transpose(out=ptr[:], in_=hr[:], identity=ident[:])
                nc.scalar.activation(hT_sb[:, t * P:(t + 1) * P], ptr[:],
                                     mybir.ActivationFunctionType.Relu,
                                     bias=b1_sb[:, 0:1])

            # ---- phase C: segment max pooling + linear head + log_softmax ----
            pooled = constp.tile([P, GPC], f32)
            for k in range(GPC):
                nc.vector.reduce_max(pooled[:, k:k + 1],
                                     hT_sb[:, gb_local[k]:gb_local[k + 1]],
                                     axis=mybir.AxisListType.X)
            pl = psa.tile([GPC, N_CLASSES], f32, tag="pl")
            nc.tensor.matmul(pl[:], lhsT=pooled[:], rhs=w2_sb[:],
                             start=True, stop=True)
            ls = constp.tile([GPC, N_CLASSES], f32)
            nc.vector.tensor_add(out=ls[:], in0=pl[:], in1=b2_sb[:])
            mx = constp.tile([GPC, 1], f32)
            nc.vector.reduce_max(mx[:], ls[:], axis=mybir.AxisListType.X)
            xm = constp.tile([GPC, N_CLASSES], f32)
            nc.vector.tensor_scalar_sub(xm[:], ls[:], mx[:, 0:1])
            ex = constp.tile([GPC, N_CLASSES], f32)
            nc.scalar.activation(ex[:], xm[:],
                                 mybir.ActivationFunctionType.Exp)
            sm = constp.tile([GPC, 1], f32)
            nc.vector.reduce_sum(sm[:], ex[:], axis=mybir.AxisListType.X)
            lg = constp.tile([GPC, 1], f32)
            nc.scalar.activation(lg[:], sm[:],
                                 mybir.ActivationFunctionType.Ln)
            of = constp.tile([GPC, N_CLASSES], f32)
            nc.vector.tensor_scalar_sub(of[:], xm[:], lg[:, 0:1])
            nc.sync.dma_start(out=out.ap()[:, :], in_=of[:])

    nc.compile()
    return nc


def _install_ntff_hook():
    """Provide antenv.axon_hooks.get_axon_ntff_profile_hook when the agent
    image lacks it (ctypes bridge into libaxon_pjrt.so, mirroring
    trn_boot._ntff_profile_via_ctypes)."""
    import contextlib
    import ctypes
    import sys
    import types
    try:
        import antenv.axon_hooks  # noqa: F401
        return
    except ImportError:
        pass
    the_hook = None
    try:
        lib = ctypes.CDLL("/opt/axon/libaxon_pjrt.so")
        if hasattr(lib, "axon_start_nrt_profile"):
            lib.axon_start_nrt_profile.argtypes = [
                ctypes.POINTER(ctypes.c_int64), ctypes.c_size_t]
            lib.axon_start_nrt_profile.restype = ctypes.c_int64
            lib.axon_stop_nrt_profile.argtypes = [ctypes.c_char_p]
            lib.axon_stop_nrt_profile.restype = ctypes.c_int64

            @contextlib.contextmanager
            def _hook(output_dir, device_ids):
                import jax
                jax.devices()
                if device_ids:
                    ids = (ctypes.c_int64 * len(device_ids))(*device_ids)
                    rc = lib.axon_start_nrt_profile(ids, len(device_ids))
                else:
                    rc = lib.axon_start_nrt_profile(None, 0)
                if rc != 0:
                    raise RuntimeError(f"axon_start_nrt_profile rc={rc}")
                try:
                    yield
                finally:
                    n = lib.axon_stop_nrt_profile(str(output_dir).encode())
                    print(f"ntff profile: {n} file(s) -> {output_dir}")

            the_hook = _hook
    except OSError:
        pass
    mod = types.ModuleType("antenv.axon_hooks")
    mod.get_axon_ntff_profile_hook = lambda: the_hook
    mod.set_axon_ntff_profile_hook = lambda h: None
    import antenv
    antenv.axon_hooks = mod
    sys.modules["antenv.axon_hooks"] = mod


def kernel(x, W1, b1, W2, b2, edge_index, batch, _trace=False, _trace_kwargs=None):
    global LAST_RESULTS
    import shutil
    from concourse import bass_utils
    if _trace:
        _install_ntff_hook()
        # no fish/S3 in this container; keep artifacts local
        bass_utils.upload_artifacts = lambda tmpdir: tmpdir
        shutil.rmtree("/tmp/gnn_neff", ignore_errors=True)

    chunks, gb_local, in_maps = _host_prep(x, W1, b1, W2, b2,
                                           edge_index, batch)
    key = (chunks, gb_local)
    nc = _PROGRAM_CACHE.get(key)
    if nc is None:
        nc = _build_program(chunks, gb_local)
        _PROGRAM_CACHE[key] = nc

    res = bass_utils.run_bass_kernel_spmd(
        nc, in_maps, core_ids=list(range(NCORES)),
        trace=_trace, tmpdir="/tmp/gnn_neff" if _trace else None,
        **(_trace_kwargs or {}))
    LAST_RESULTS = res
    return np.concatenate([res.results[c]["out"] for c in range(NCORES)],
                          axis=0)


# revision 21
# speedup vs baseline: 1.0072x; 1.0072x over previous
"""GCN message-passing kernel for Trainium2, 8 NeuronCores.

Model (see reference):
    h   = relu(GCNConv(x, edge_index; W1, b1))      # [N, 128]
    p   = segment_max(h, batch, 128 graphs)          # [128, 128]
    out = log_softmax(p @ W2 + b2)                   # [128, 2]

GCNConv with self loops and symmetric norm decomposes as
    out = D^-1/2 * A * D^-1/2 * (x @ W1) + b1
so we compute H' = dinv * (x@W1) row-scaled, aggregate H'[src] into dst with
an unweighted segment-sum, then row-scale by dinv[dst] again.

Distribution (8 cores):
  * nodes row-sharded contiguously: core c owns nodes [c*6250, (c+1)*6250)
  * core c computes its H' slice; two chunked AllGathers (first 28 tiles,
    then the last 21) so the second collective overlaps the first gathers.
  * edges sharded by dst ownership (sorted by dst on host); each core
    aggregates its own dst rows:  for each 128-dst-node tile, gather
    H'[src] rows via indirect DMA, build a one-hot selection matrix from
    the dst indices (iota == dstrel), and matmul-accumulate into PSUM.
    Edges are banded by 64-dst group within the tile so the one-hot is
    [slots, 64] per chunk (half the DVE area; DVE 2-port ops lock the Q7
    descriptor generator out of SBUF, so DVE time is Pool stall time).
    Self-loop edges are NOT gathered: the local H' tile is added with
    identity matmuls per 64-band instead.
  * graph boundaries align with the node shard (6250 nodes = exactly 16
    graphs per core), so segment_max + head are fully core-local.
  * final [16,2] per-core outputs are concatenated on host.
"""

import math

import numpy as np

N_NODES = 50000
N_EDGES = 1600000
N_FEAT = 512
N_HID = 128
N_CLASSES = 2
NUM_GRAPHS = 128
NCORES = 8
NPC = N_NODES // NCORES          # 6250 nodes per core
GPC = NUM_GRAPHS // NCORES       # 16 graphs per core
P = 128
NT = (NPC + P - 1) // P          # 49 dst tiles per core (last tile 106 valid)
KF = N_FEAT // P                 # 4 K-chunks for x @ W1
GT = 7                           # tiles per phase-A DMA group (7*7 = 49)
NTA = 28                         # tiles in collective piece A (4 groups)
NTB = NT - NTA                   # 21 tiles in piece B (3 groups)
RA = NTA * P                     # 3584 local rows in piece A
RB = NTB * P                     # 2688 local rows in piece B
W = 64                           # dst band width (2 bands per tile)
NB = P // W                      # 2 bands

_PROGRAM_CACHE: dict = {}
LAST_RESULTS = None              # BassKernelResults of the most recent run


def _host_prep(x, W1, b1, W2, b2, edge_index, batch):
    """All integer/index preprocessing + input shard construction."""
    x = np.asarray(x, dtype=np.float32)
    W1 = np.asarray(W1, dtype=np.float32)
    b1 = np.asarray(b1, dtype=np.float32)
    W2 = np.asarray(W2, dtype=np.float32)
    b2 = np.asarray(b2, dtype=np.float32)
    ei = np.asarray(edge_index)
    batch = np.asarray(batch)

    src = ei[0].astype(np.int32)
    dst = ei[1].astype(np.int32)

    # in-degree (with self loops); float input to the device rsqrt.
    # Self loops are handled by identity matmuls on the local H' tile,
    # so they are excluded from the gathered edge list (but kept in deg).
    deg = (np.bincount(dst, minlength=N_NODES) + 1).astype(np.float32)

    # gather table row: piece A = local rows [0, RA) of every core packed
    # first (rows c*RA + l), piece B = local rows [RA, NPC) (rows
    # 8*RA + c*RB + (l-RA)).  "half" h is the piece id; both pieces'
    # row spaces stay below 32768 so int16 gather indices reach them.
    lcl = src % NPC
    half_of = (lcl >= RA).astype(np.int64)
    row_of = np.where(half_of == 0,
                      (src // NPC) * RA + lcl,
                      (src // NPC) * RB + (lcl - RA))
    tid = (dst // NPC) * NT + (dst % NPC) // P               # global dst tile
    # sort by (dst tile, table piece, src row): the src-row order makes the
    # SDMA m2s reads walk monotonically increasing HBM addresses (row-buffer
    # locality); slot order within a tile is free (one-hot maps slot->dst).
    order = np.lexsort((row_of, half_of, tid))
    row_s = row_of[order]
    dst_s = dst[order]
    key_s = tid[order] * 2 + half_of[order]

    # graph boundaries from the actual batch tensor; must align to the shard
    gbs = np.searchsorted(batch, np.arange(NUM_GRAPHS + 1))
    gb_local = gbs[:GPC + 1].astype(np.int64).copy()
    for c in range(NCORES):
        seg = gbs[c * GPC:(c + 1) * GPC + 1] - c * NPC
        assert np.array_equal(seg, gb_local), "graph/node shard misalignment"

    # per (core, tile, half) edge counts -> common chunk schedule
    # (tid encodes the core, so key_s separates cores already)
    cell_ofs = np.searchsorted(key_s, np.arange(NCORES * NT * 2 + 1))
    cnt = (cell_ofs[1:] - cell_ofs[:-1]).reshape(NCORES, NT, 2)
    chunks = -(-cnt.max(axis=0) // P)                        # [NT, 2]
    q0 = np.zeros((NT, 2), dtype=np.int64)
    ct0 = np.zeros((NT, 2), dtype=np.int64)
    acc_c = acc_q = 0
    for t in range(NT):
        for h in range(2):
            q0[t, h] = acc_q
            ct0[t, h] = acc_c
            acc_c += chunks[t, h]
            acc_q += int(chunks[t, h]) * 8                   # 128/16 cols16
    ctot = int(acc_c)
    qtot = int(acc_q)

    gidx = np.zeros((NCORES, P, qtot), dtype=np.int16)
    dstr = np.full((NCORES, P, ctot), 255.0, dtype=np.float16)
    for c in range(NCORES):
        for t in range(NT):
            for h in range(2):
                i = (c * NT + t) * 2 + h
                e0 = cell_ofs[i]
                n = int(cnt[c, t, h])
                K = int(chunks[t, h]) * P                    # padded len
                if K == 0:
                    continue
                idx = np.zeros(K, dtype=np.int16)            # pad: row 0
                idx[:n] = row_s[e0:e0 + n].astype(np.int16)
                gidx[c, :, q0[t, h]:q0[t, h] + K // 16] = \
                    np.tile(idx.reshape(K // 16, 16).T, (8, 1))
                if n:
                    s = np.arange(n)
                    dstr[c, s % P, ct0[t, h] + s // P] = \
                        (dst_s[e0:e0 + n]
                         - (c * NPC + t * P)).astype(np.float16)

    # x transposed + padded to the tile grid; deg per-core in [128, NT] layout
    xT = np.ascontiguousarray(x.T).astype(np.float16)       # [512, 50000]
    xT_pad = np.zeros((NCORES, N_FEAT, NT * P), dtype=np.float16)
    deg_cols = np.ones((NCORES, P, NT), dtype=np.float32)
    for c in range(NCORES):
        xT_pad[c, :, :NPC] = xT[:, c * NPC:(c + 1) * NPC]
        d = deg[c * NPC:(c + 1) * NPC]                      # [6250]
        dp = np.ones(NT * P, dtype=np.float32)
        dp[:NPC] = d
        deg_cols[c] = dp.reshape(NT, P).T

    iota_mat = np.tile(np.arange(P, dtype=np.float16), (P, 1))
    ident_f16 = np.eye(P, dtype=np.float16)
    b1_col = b1.reshape(N_HID, 1).astype(np.float32)
    b2_mat = np.tile(b2[None, :], (GPC, 1)).astype(np.float32)

    in_maps = []
    for c in range(NCORES):
        in_maps.append({
            "xT": xT_pad[c],
            "w1": W1.astype(np.float16),
            "b1c": b1_col,
            "w2": W2,
            "b2m": b2_mat,
            "degc": deg_cols[c],
            "gidx": gidx[c],
            "dstr": dstr[c],
            "iot": iota_mat,
            "idn": ident_f16,
        })
    chunks_key = tuple(tuple(int(v) for v in row) for row in chunks)
    return chunks_key, tuple(int(v) for v in gb_local), in_maps


def _build_program(chunks, gb_local):
    import concourse.bacc as bacc
    import concourse.bass as bass
    import concourse.mybir as mybir
    import concourse.tile as tile
    from concourse.masks import make_identity

    f32 = mybir.dt.float32
    f16 = mybir.dt.float16
    i16 = mybir.dt.int16
    # chunks: [NT][2]; offsets mirror _host_prep
    q0 = [[0, 0] for _ in range(NT)]
    ct0 = [[0, 0] for _ in range(NT)]
    acc_c = acc_q = 0
    for t in range(NT):
        for h in range(2):
            q0[t][h] = acc_q
            ct0[t][h] = acc_c
            acc_c += chunks[t][h]
            acc_q += chunks[t][h] * 8
    ctot = acc_c
    qtot = acc_q

    nc = bacc.Bacc("TRN2", target_bir_lowering=False, debug=False,
                   num_devices=NCORES, num_swdge_queues=4,
                   dynamic_dma_scratch_size=49152)

    xT = nc.dram_tensor("xT", [N_FEAT, NT * P], f16, kind="ExternalInput")
    w1 = nc.dram_tensor("w1", [N_FEAT, N_HID], f16, kind="ExternalInput")
    b1c = nc.dram_tensor("b1c", [N_HID, 1], f32, kind="ExternalInput")
    w2 = nc.dram_tensor("w2", [N_HID, N_CLASSES], f32, kind="ExternalInput")
    b2m = nc.dram_tensor("b2m", [GPC, N_CLASSES], f32, kind="ExternalInput")
    degc = nc.dram_tensor("degc", [P, NT], f32, kind="ExternalInput")
    gidx = nc.dram_tensor("gidx", [P, qtot], i16, kind="ExternalInput")
    dstr = nc.dram_tensor("dstr", [P, ctot], f16, kind="ExternalInput")
    iot = nc.dram_tensor("iot", [P, P], f16, kind="ExternalInput")
    idn = nc.dram_tensor("idn", [P, P], f16, kind="ExternalInput")
    out = nc.dram_tensor("out", [GPC, N_CLASSES], f32, kind="ExternalOutput")

    with tile.TileContext(nc) as tc:
        with (
            tc.tile_pool(name="const", bufs=1) as constp,
            tc.tile_pool(name="xg", bufs=2) as xgp,
            tc.tile_pool(name="oh", bufs=3) as ohp,
            tc.tile_pool(name="gg", bufs=6) as ggp,
            tc.tile_pool(name="hr", bufs=3) as hrp,
            tc.tile_pool(name="ps_h", bufs=2, space="PSUM") as psh,
            tc.tile_pool(name="ps_a", bufs=2, space="PSUM") as psa,
            tc.tile_pool(name="ps_t", bufs=2, space="PSUM") as pst,
            tc.tile_pool(name="dram", bufs=1, space="DRAM") as dramp,
        ):
            # ---- constants / persistent state ----
            w1_sb = constp.tile([P, N_FEAT], f16)            # [p, (k n)]
            nc.sync.dma_start(
                out=w1_sb[:].rearrange("p (k n) -> p k n", k=KF),
                in_=w1.ap().rearrange("(k p) n -> p k n", p=P))
            iota_sb = constp.tile([P, P], f16)
            nc.sync.dma_start(out=iota_sb[:], in_=iot.ap())
            idn_sb = constp.tile([P, P], f16)
            nc.sync.dma_start(out=idn_sb[:], in_=idn.ap())
            b1_sb = constp.tile([N_HID, 1], f32)
            nc.sync.dma_start(out=b1_sb[:], in_=b1c.ap())
            w2_sb = constp.tile([N_HID, N_CLASSES], f32)
            nc.sync.dma_start(out=w2_sb[:], in_=w2.ap())
            b2_sb = constp.tile([GPC, N_CLASSES], f32)
            nc.sync.dma_start(out=b2_sb[:], in_=b2m.ap())
            ident = constp.tile([P, P], f32)
            make_identity(nc, ident[:])

            deg_sb = constp.tile([P, NT], f32)
            nc.sync.dma_start(out=deg_sb[:], in_=degc.ap())
            rec_sb = constp.tile([P, NT], f32)
            nc.vector.reciprocal(rec_sb[:], deg_sb[:])
            dinv_sb = constp.tile([P, NT], f32)
            nc.scalar.activation(dinv_sb[:], rec_sb[:],
                                 mybir.ActivationFunctionType.Sqrt)

            gidx_sb = constp.tile([P, qtot], i16)
            nc.sync.dma_start(out=gidx_sb[:], in_=gidx.ap())
            dstr_sb = constp.tile([P, ctot], f16)
            nc.sync.dma_start(out=dstr_sb[:], in_=dstr.ap())

            hp_all = constp.tile([P, NT * P], f16)           # local H' tiles
            hT_sb = constp.tile([P, NT * P], f16)            # [hid, node]

            cc_in = dramp.tile([NT * P, N_HID], f16)
            cc_fa = dramp.tile([NCORES * RA, N_HID], f16, addr_space="Shared")
            cc_fb = dramp.tile([NCORES * RB, N_HID], f16, addr_space="Shared")

            # ---- phase A: H' = dinv * (x @ W1), per 128-node tile ----
            # xT loads and cc_in stores batched in groups of GT tiles.
            for g in range(NT // GT):
                xg_t = xgp.tile([P, KF * GT * P], f16, tag="xg")
                nc.sync.dma_start(
                    out=xg_t[:].rearrange("p (k c) -> p k c", k=KF),
                    in_=xT.ap()[:, g * GT * P:(g + 1) * GT * P]
                        .rearrange("(k p) c -> p k c", p=P))
                for ti in range(GT):
                    t = g * GT + ti
                    ph = psh.tile([P, N_HID], f32, tag="ph")
                    for k in range(KF):
                        nc.tensor.matmul(
                            ph[:],
                            lhsT=xg_t[:, (k * GT + ti) * P:(k * GT + ti + 1) * P],
                            rhs=w1_sb[:, k * P:(k + 1) * P],
                            start=(k == 0), stop=(k == KF - 1))
                    nc.scalar.activation(hp_all[:, t * P:(t + 1) * P], ph[:],
                                         mybir.ActivationFunctionType.Copy,
                                         scale=dinv_sb[:, t:t + 1])
                nc.sync.dma_start(
                    out=cc_in[g * GT * P:(g + 1) * GT * P, :]
                        .rearrange("(T p) m -> p T m", p=P),
                    in_=hp_all[:, g * GT * P:(g + 1) * GT * P]
                        .rearrange("p (T m) -> p T m", m=P))
                if g == NTA // GT - 1:
                    # piece A complete -> overlap its AllGather with the
                    # rest of phase A and start gathers under collective B
                    nc.gpsimd.collective_compute(
                        "AllGather", mybir.AluOpType.bypass,
                        replica_groups=[list(range(NCORES))],
                        ins=[cc_in[0:RA, :]],
                        outs=[cc_fa[:, :]],
                    )
            nc.gpsimd.collective_compute(
                "AllGather", mybir.AluOpType.bypass,
                replica_groups=[list(range(NCORES))],
                ins=[cc_in[RA:NT * P, :]],
                outs=[cc_fb[:, :]],
            )

            # ---- phase B: aggregate per dst tile ----
            # software-pipelined: piece-A gathers run LA tiles ahead of the
            # piece-B gathers + compute, so the early piece-A work hides
            # collective B's latency and keeps more DMA queues in flight.
            qn = 0
            LA = 2
            g_tiles = [None] * NT

            def emit_gathers(t, h, g_t, goff):
                # <=1024-idx sub-gathers: single_packet coalesces each into
                # one 64-desc packet per engine, which the SDMA drains
                # pipelined (~4x the per-descriptor rate of
                # descriptor-per-packet mode; >64-desc packets hang).
                nonlocal qn
                ch = chunks[t][h]
                coff = 0
                while coff < ch:
                    sub = min(8, ch - coff)
                    K = sub * P
                    qo = q0[t][h] + coff * 8
                    nc.gpsimd.dma_gather(
                        out_ap=g_t[:, (goff + coff) * P:
                                   (goff + coff + sub) * P]
                            .rearrange("p (j f) -> p j f", f=N_HID),
                        in_ap=(cc_fa if h == 0 else cc_fb)[:, :],
                        idxs_ap=gidx_sb[:, qo:qo + K // 16],
                        num_idxs=K,
                        num_idxs_reg=K,
                        elem_size=N_HID,
                        single_packet=True,
                        queue_num=qn % 4,
                    )
                    qn += 1
                    coff += sub

            for i in range(NT + LA):
                if i < NT:
                    cn_i = chunks[i][0] + chunks[i][1]
                    g_tiles[i] = ggp.tile([P, cn_i * P], f16, tag="g",
                                          name=f"g_t{i}")
                    emit_gathers(i, 0, g_tiles[i], 0)
                if i < LA:
                    continue
                t = i - LA
                cn = chunks[t][0] + chunks[t][1]
                g_t = g_tiles[t]
                g_tiles[t] = None
                emit_gathers(t, 1, g_t, chunks[t][0])
                oh_t = ohp.tile([P, cn * P], f16, tag="oh")
                # one-hot: oh[p, (c n)] = (iota[n] == dstrel[p, c])
                nc.vector.tensor_tensor(
                    out=oh_t[:].rearrange("p (c n) -> p c n", n=P),
                    in0=iota_sb[:].rearrange("p (one n) -> p one n", one=1)
                        .to_broadcast([P, cn, P]),
                    in1=dstr_sb[:, ct0[t][0]:ct0[t][0] + cn]
                        .rearrange("p (c one) -> p c one", one=1)
                        .to_broadcast([P, cn, P]),
                    op=mybir.AluOpType.is_equal,
                )
                pa = psa.tile([P, N_HID], f32, tag="pa")
                for j in range(cn):
                    nc.tensor.matmul(pa[:],
                                     lhsT=oh_t[:, j * P:(j + 1) * P],
                                     rhs=g_t[:, j * P:(j + 1) * P],
                                     start=(j == 0), stop=False)
                # self loops: += I^T @ H'_local  (H' rows for this tile)
                nc.tensor.matmul(pa[:], lhsT=idn_sb[:],
                                 rhs=hp_all[:, t * P:(t + 1) * P],
                                 start=False, stop=True)
                # epilogue on ACT + PE only (DVE stalls badly while the Q7
                # gather ucode is hammering SBUF): scale rows by dinv[dst],
                # transpose to [hid, node], then relu(x + b1) with b1 as the
                # per-partition ACT bias in the transposed layout.
                hr = hrp.tile([P, N_HID], f32, tag="hr")
                nc.scalar.activation(hr[:], pa[:],
                                     mybir.ActivationFunctionType.Copy,
                                     scale=dinv_sb[:, t:t + 1])
                ptr = pst.tile([P, P], f32, tag="pt")
                nc.tensor.transpose(out=ptr[:], in_=hr[:], identity=ident[:])
                nc.scalar.activation(hT_sb[:, t * P:(t + 1) * P], ptr[:],
                                     mybir.ActivationFunctionType.Relu,
                                     bias=b1_sb[:, 0:1])

            # ---- phase C: segment max pooling + linear head + log_softmax ----
            pooled = constp.tile([P, GPC], f32)
            for k in range(GPC):
                nc.vector.reduce_max(pooled[:, k:k + 1],
                                     hT_sb[:, gb_local[k]:gb_local[k + 1]],
                                     axis=mybir.AxisListType.X)
            pl = psa.tile([GPC, N_CLASSES], f32, tag="pl")
            nc.tensor.matmul(pl[:], lhsT=pooled[:], rhs=w2_sb[:],
                             start=True, stop=True)
            ls = constp.tile([GPC, N_CLASSES], f32)
            nc.vector.tensor_add(out=ls[:], in0=pl[:], in1=b2_sb[:])
            mx = constp.tile([GPC, 1], f32)
            nc.vector.reduce_max(mx[:], ls[:], axis=mybir.AxisListType.X)
            xm = constp.tile([GPC, N_CLASSES], f32)
            nc.vector.tensor_scalar_sub(xm[:], ls[:], mx[:, 0:1])
            ex = constp.tile([GPC, N_CLASSES], f32)
            nc.scalar.activation(ex[:], xm[:],
                                 mybir.ActivationFunctionType.Exp)
            sm = constp.tile([GPC, 1], f32)
            nc.vector.reduce_sum(sm[:], ex[:], axis=mybir.AxisListType.X)
            lg = constp.tile([GPC, 1], f32)
            nc.scalar.activation(lg[:], sm[:],
                                 mybir.ActivationFunctionType.Ln)
            of = constp.tile([GPC, N_CLASSES], f32)
            nc.vector.tensor_scalar_sub(of[:], xm[:], lg[:, 0:1])
            nc.sync.dma_start(out=out.ap()[:, :], in_=of[:])

    nc.compile()
    return nc


def _install_ntff_hook():
    """Provide antenv.axon_hooks.get_axon_ntff_profile_hook when the agent
    image lacks it (ctypes bridge into libaxon_pjrt.so, mirroring
    trn_boot._ntff_profile_via_ctypes)."""
    import contextlib
    import ctypes
    import sys
    import types
    try:
        import antenv.axon_hooks  # noqa: F401
        return
    except ImportError:
        pass
    the_hook = None
    try:
        lib = ctypes.CDLL("/opt/axon/libaxon_pjrt.so")
        if hasattr(lib, "axon_start_nrt_profile"):
            lib.axon_start_nrt_profile.argtypes = [
                ctypes.POINTER(ctypes.c_int64), ctypes.c_size_t]
            lib.axon_start_nrt_profile.restype = ctypes.c_int64
            lib.axon_stop_nrt_profile.argtypes = [ctypes.c_char_p]
            lib.axon_stop_nrt_profile.restype = ctypes.c_int64

            @contextlib.contextmanager
            def _hook(output_dir, device_ids):
                import jax
                jax.devices()
                if device_ids:
                    ids = (ctypes.c_int64 * len(device_ids))(*device_ids)
                    rc = lib.axon_start_nrt_profile(ids, len(device_ids))
                else:
                    rc = lib.axon_start_nrt_profile(None, 0)
                if rc != 0:
                    raise RuntimeError(f"axon_start_nrt_profile rc={rc}")
                try:
                    yield
                finally:
                    n = lib.axon_stop_nrt_profile(str(output_dir).encode())
                    print(f"ntff profile: {n} file(s) -> {output_dir}")

            the_hook = _hook
    except OSError:
        pass
    mod = types.ModuleType("antenv.axon_hooks")
    mod.get_axon_ntff_profile_hook = lambda: the_hook
    mod.set_axon_ntff_profile_hook = lambda h: None
    import antenv
    antenv.axon_hooks = mod
    sys.modules["antenv.axon_hooks"] = mod


def kernel(x, W1, b1, W2, b2, edge_index, batch, _trace=False, _trace_kwargs=None):
    global LAST_RESULTS
    import shutil
    from concourse import bass_utils
    if _trace:
        _install_ntff_hook()
        # no fish/S3 in this container; keep artifacts local
        bass_utils.upload_artifacts = lambda tmpdir: tmpdir
        shutil.rmtree("/tmp/gnn_neff", ignore_errors=True)

    chunks, gb_local, in_maps = _host_prep(x, W1, b1, W2, b2,
                                           edge_index, batch)
    key = (chunks, gb_local)
    nc = _PROGRAM_CACHE.get(key)
    if nc is None:
        nc = _build_program(chunks, gb_local)
        _PROGRAM_CACHE[key] = nc

    res = bass_utils.run_bass_kernel_spmd(
        nc, in_maps, core_ids=list(range(NCORES)),
        trace=_trace, tmpdir="/tmp/gnn_neff" if _trace else None,
        **(_trace_kwargs or {}))
    LAST_RESULTS = res
    return np.concatenate([res.results[c]["out"] for c in range(NCORES)],
                          axis=0)


# revision 23
# speedup vs baseline: 1.1349x; 1.1268x over previous
"""GCN message-passing kernel for Trainium2, 8 NeuronCores.

Model (see reference):
    h   = relu(GCNConv(x, edge_index; W1, b1))      # [N, 128]
    p   = segment_max(h, batch, 128 graphs)          # [128, 128]
    out = log_softmax(p @ W2 + b2)                   # [128, 2]

GCNConv with self loops and symmetric norm decomposes as
    out = D^-1/2 * A * D^-1/2 * (x @ W1) + b1
so we compute H' = dinv * (x@W1) row-scaled, aggregate H'[src] into dst with
an unweighted segment-sum, then row-scale by dinv[dst] again.

Distribution (8 cores):
  * nodes row-sharded contiguously: core c owns nodes [c*6250, (c+1)*6250)
  * core c computes its H' slice; two chunked AllGathers (first 28 tiles,
    then the last 21) so the second collective overlaps the first gathers.
  * edges sharded by dst ownership (sorted by dst on host); each core
    aggregates its own dst rows:  for each 128-dst-node tile, gather
    H'[src] rows via indirect DMA, build a one-hot selection matrix from
    the dst indices (iota == dstrel), and matmul-accumulate into PSUM.
    Edges are banded by 64-dst group within the tile so the one-hot is
    [slots, 64] per chunk (half the DVE area; DVE 2-port ops lock the Q7
    descriptor generator out of SBUF, so DVE time is Pool stall time).
    Self-loop edges are NOT gathered: the local H' tile is added with
    identity matmuls per 64-band instead.
  * graph boundaries align with the node shard (6250 nodes = exactly 16
    graphs per core), so segment_max + head are fully core-local.
  * final [16,2] per-core outputs are concatenated on host.
"""

import math

import numpy as np

N_NODES = 50000
N_EDGES = 1600000
N_FEAT = 512
N_HID = 128
N_CLASSES = 2
NUM_GRAPHS = 128
NCORES = 8
NPC = N_NODES // NCORES          # 6250 nodes per core
GPC = NUM_GRAPHS // NCORES       # 16 graphs per core
P = 128
NT = (NPC + P - 1) // P          # 49 dst tiles per core (last tile 106 valid)
KF = N_FEAT // P                 # 4 K-chunks for x @ W1
GT = 7                           # tiles per phase-A DMA group (7*7 = 49)
NTA = 28                         # tiles in collective piece A (4 groups)
NTB = NT - NTA                   # 21 tiles in piece B (3 groups)
RA = NTA * P                     # 3584 local rows in piece A
RB = NTB * P                     # 2688 local rows in piece B
W = 64                           # dst band width (2 bands per tile)
NB = P // W                      # 2 bands

_PROGRAM_CACHE: dict = {}
LAST_RESULTS = None              # BassKernelResults of the most recent run


def _host_prep(x, W1, b1, W2, b2, edge_index, batch):
    """All integer/index preprocessing + input shard construction."""
    x = np.asarray(x, dtype=np.float32)
    W1 = np.asarray(W1, dtype=np.float32)
    b1 = np.asarray(b1, dtype=np.float32)
    W2 = np.asarray(W2, dtype=np.float32)
    b2 = np.asarray(b2, dtype=np.float32)
    ei = np.asarray(edge_index)
    batch = np.asarray(batch)

    src = ei[0].astype(np.int32)
    dst = ei[1].astype(np.int32)

    # in-degree (with self loops); float input to the device rsqrt.
    # Self loops are handled by identity matmuls on the local H' tile,
    # so they are excluded from the gathered edge list (but kept in deg).
    deg = (np.bincount(dst, minlength=N_NODES) + 1).astype(np.float32)

    # gather table row: piece A = local rows [0, RA) of every core packed
    # first (rows c*RA + l), piece B = local rows [RA, NPC) (rows
    # 8*RA + c*RB + (l-RA)).  "half" h is the piece id; both pieces'
    # row spaces stay below 32768 so int16 gather indices reach them.
    lcl = src % NPC
    half_of = (lcl >= RA).astype(np.int64)
    row_of = np.where(half_of == 0,
                      (src // NPC) * RA + lcl,
                      (src // NPC) * RB + (lcl - RA))
    tid = (dst // NPC) * NT + (dst % NPC) // P               # global dst tile
    # sort by (dst tile, table piece, src row): the src-row order makes the
    # SDMA m2s reads walk monotonically increasing HBM addresses (row-buffer
    # locality); slot order within a tile is free (one-hot maps slot->dst).
    order = np.lexsort((row_of, half_of, tid))
    row_s = row_of[order]
    dst_s = dst[order]
    key_s = tid[order] * 2 + half_of[order]

    # graph boundaries from the actual batch tensor; must align to the shard
    gbs = np.searchsorted(batch, np.arange(NUM_GRAPHS + 1))
    gb_local = gbs[:GPC + 1].astype(np.int64).copy()
    for c in range(NCORES):
        seg = gbs[c * GPC:(c + 1) * GPC + 1] - c * NPC
        assert np.array_equal(seg, gb_local), "graph/node shard misalignment"

    # per (core, tile, half) edge counts -> common chunk schedule
    # (tid encodes the core, so key_s separates cores already)
    cell_ofs = np.searchsorted(key_s, np.arange(NCORES * NT * 2 + 1))
    cnt = (cell_ofs[1:] - cell_ofs[:-1]).reshape(NCORES, NT, 2)
    chunks = -(-cnt.max(axis=0) // P)                        # [NT, 2]
    q0 = np.zeros((NT, 2), dtype=np.int64)
    ct0 = np.zeros((NT, 2), dtype=np.int64)
    acc_c = acc_q = 0
    for t in range(NT):
        for h in range(2):
            q0[t, h] = acc_q
            ct0[t, h] = acc_c
            acc_c += chunks[t, h]
            acc_q += int(chunks[t, h]) * 8                   # 128/16 cols16
    ctot = int(acc_c)
    qtot = int(acc_q)

    gidx = np.zeros((NCORES, P, qtot), dtype=np.int16)
    dstr = np.full((NCORES, P, ctot), 255.0, dtype=np.float16)
    for c in range(NCORES):
        for t in range(NT):
            for h in range(2):
                i = (c * NT + t) * 2 + h
                e0 = cell_ofs[i]
                n = int(cnt[c, t, h])
                K = int(chunks[t, h]) * P                    # padded len
                if K == 0:
                    continue
                idx = np.zeros(K, dtype=np.int16)            # pad: row 0
                idx[:n] = row_s[e0:e0 + n].astype(np.int16)
                gidx[c, :, q0[t, h]:q0[t, h] + K // 16] = \
                    np.tile(idx.reshape(K // 16, 16).T, (8, 1))
                if n:
                    s = np.arange(n)
                    dstr[c, s % P, ct0[t, h] + s // P] = \
                        (dst_s[e0:e0 + n]
                         - (c * NPC + t * P)).astype(np.float16)

    # x transposed + padded to the tile grid; deg per-core in [128, NT] layout
    xT = np.ascontiguousarray(x.T).astype(np.float16)       # [512, 50000]
    xT_pad = np.zeros((NCORES, N_FEAT, NT * P), dtype=np.float16)
    deg_cols = np.ones((NCORES, P, NT), dtype=np.float32)
    for c in range(NCORES):
        xT_pad[c, :, :NPC] = xT[:, c * NPC:(c + 1) * NPC]
        d = deg[c * NPC:(c + 1) * NPC]                      # [6250]
        dp = np.ones(NT * P, dtype=np.float32)
        dp[:NPC] = d
        deg_cols[c] = dp.reshape(NT, P).T

    iota_mat = np.tile(np.arange(P, dtype=np.float16), (P, 1))
    ident_f16 = np.eye(P, dtype=np.float16)
    b1_col = b1.reshape(N_HID, 1).astype(np.float32)
    b2_mat = np.tile(b2[None, :], (GPC, 1)).astype(np.float32)

    in_maps = []
    for c in range(NCORES):
        in_maps.append({
            "xT": xT_pad[c],
            "w1": W1.astype(np.float16),
            "b1c": b1_col,
            "w2": W2,
            "b2m": b2_mat,
            "degc": deg_cols[c],
            "gidx": gidx[c],
            "dstr": dstr[c],
            "iot": iota_mat,
            "idn": ident_f16,
        })
    chunks_key = tuple(tuple(int(v) for v in row) for row in chunks)
    return chunks_key, tuple(int(v) for v in gb_local), in_maps


def _build_program(chunks, gb_local):
    import concourse.bacc as bacc
    import concourse.bass as bass
    import concourse.mybir as mybir
    import concourse.tile as tile
    from concourse.masks import make_identity

    f32 = mybir.dt.float32
    f16 = mybir.dt.float16
    i16 = mybir.dt.int16
    # chunks: [NT][2]; offsets mirror _host_prep
    q0 = [[0, 0] for _ in range(NT)]
    ct0 = [[0, 0] for _ in range(NT)]
    acc_c = acc_q = 0
    for t in range(NT):
        for h in range(2):
            q0[t][h] = acc_q
            ct0[t][h] = acc_c
            acc_c += chunks[t][h]
            acc_q += chunks[t][h] * 8
    ctot = acc_c
    qtot = acc_q

    nc = bacc.Bacc("TRN2", target_bir_lowering=False, debug=False,
                   num_devices=NCORES, num_swdge_queues=4,
                   dynamic_dma_scratch_size=49152)

    xT = nc.dram_tensor("xT", [N_FEAT, NT * P], f16, kind="ExternalInput")
    w1 = nc.dram_tensor("w1", [N_FEAT, N_HID], f16, kind="ExternalInput")
    b1c = nc.dram_tensor("b1c", [N_HID, 1], f32, kind="ExternalInput")
    w2 = nc.dram_tensor("w2", [N_HID, N_CLASSES], f32, kind="ExternalInput")
    b2m = nc.dram_tensor("b2m", [GPC, N_CLASSES], f32, kind="ExternalInput")
    degc = nc.dram_tensor("degc", [P, NT], f32, kind="ExternalInput")
    gidx = nc.dram_tensor("gidx", [P, qtot], i16, kind="ExternalInput")
    dstr = nc.dram_tensor("dstr", [P, ctot], f16, kind="ExternalInput")
    iot = nc.dram_tensor("iot", [P, P], f16, kind="ExternalInput")
    idn = nc.dram_tensor("idn", [P, P], f16, kind="ExternalInput")
    out = nc.dram_tensor("out", [GPC, N_CLASSES], f32, kind="ExternalOutput")

    with tile.TileContext(nc) as tc:
        with (
            tc.tile_pool(name="const", bufs=1) as constp,
            tc.tile_pool(name="xg", bufs=2) as xgp,
            tc.tile_pool(name="oh", bufs=3) as ohp,
            tc.tile_pool(name="gg", bufs=6) as ggp,
            tc.tile_pool(name="hr", bufs=3) as hrp,
            tc.tile_pool(name="ps_h", bufs=2, space="PSUM") as psh,
            tc.tile_pool(name="ps_a", bufs=2, space="PSUM") as psa,
            tc.tile_pool(name="ps_t", bufs=2, space="PSUM") as pst,
            tc.tile_pool(name="dram", bufs=1, space="DRAM") as dramp,
        ):
            # ---- constants / persistent state ----
            w1_sb = constp.tile([P, N_FEAT], f16)            # [p, (k n)]
            nc.sync.dma_start(
                out=w1_sb[:].rearrange("p (k n) -> p k n", k=KF),
                in_=w1.ap().rearrange("(k p) n -> p k n", p=P))
            iota_sb = constp.tile([P, P], f16)
            nc.sync.dma_start(out=iota_sb[:], in_=iot.ap())
            idn_sb = constp.tile([P, P], f16)
            nc.sync.dma_start(out=idn_sb[:], in_=idn.ap())
            b1_sb = constp.tile([N_HID, 1], f32)
            nc.sync.dma_start(out=b1_sb[:], in_=b1c.ap())
            w2_sb = constp.tile([N_HID, N_CLASSES], f32)
            nc.sync.dma_start(out=w2_sb[:], in_=w2.ap())
            b2_sb = constp.tile([GPC, N_CLASSES], f32)
            nc.sync.dma_start(out=b2_sb[:], in_=b2m.ap())
            ident = constp.tile([P, P], f32)
            make_identity(nc, ident[:])

            deg_sb = constp.tile([P, NT], f32)
            nc.sync.dma_start(out=deg_sb[:], in_=degc.ap())
            rec_sb = constp.tile([P, NT], f32)
            nc.vector.reciprocal(rec_sb[:], deg_sb[:])
            dinv_sb = constp.tile([P, NT], f32)
            nc.scalar.activation(dinv_sb[:], rec_sb[:],
                                 mybir.ActivationFunctionType.Sqrt)

            gidx_sb = constp.tile([P, qtot], i16)
            nc.sync.dma_start(out=gidx_sb[:], in_=gidx.ap())
            dstr_sb = constp.tile([P, ctot], f16)
            nc.sync.dma_start(out=dstr_sb[:], in_=dstr.ap())

            hp_all = constp.tile([P, NT * P], f16)           # local H' tiles
            hT_sb = constp.tile([P, NT * P], f16)            # [hid, node]

            cc_in = dramp.tile([NT * P, N_HID], f16)
            cc_fa = dramp.tile([NCORES * RA, N_HID], f16, addr_space="Shared")
            cc_fb = dramp.tile([NCORES * RB, N_HID], f16, addr_space="Shared")

            # ---- phase A: H' = dinv * (x @ W1), per 128-node tile ----
            # xT loads and cc_in stores batched in groups of GT tiles.
            for g in range(NT // GT):
                xg_t = xgp.tile([P, KF * GT * P], f16, tag="xg")
                nc.sync.dma_start(
                    out=xg_t[:].rearrange("p (k c) -> p k c", k=KF),
                    in_=xT.ap()[:, g * GT * P:(g + 1) * GT * P]
                        .rearrange("(k p) c -> p k c", p=P))
                for ti in range(GT):
                    t = g * GT + ti
                    ph = psh.tile([P, N_HID], f32, tag="ph")
                    for k in range(KF):
                        nc.tensor.matmul(
                            ph[:],
                            lhsT=xg_t[:, (k * GT + ti) * P:(k * GT + ti + 1) * P],
                            rhs=w1_sb[:, k * P:(k + 1) * P],
                            start=(k == 0), stop=(k == KF - 1))
                    nc.scalar.activation(hp_all[:, t * P:(t + 1) * P], ph[:],
                                         mybir.ActivationFunctionType.Copy,
                                         scale=dinv_sb[:, t:t + 1])
                nc.sync.dma_start(
                    out=cc_in[g * GT * P:(g + 1) * GT * P, :]
                        .rearrange("(T p) m -> p T m", p=P),
                    in_=hp_all[:, g * GT * P:(g + 1) * GT * P]
                        .rearrange("p (T m) -> p T m", m=P))
                if g == NTA // GT - 1:
                    # piece A complete -> overlap its AllGather with the
                    # rest of phase A and start gathers under collective B
                    nc.gpsimd.collective_compute(
                        "AllGather", mybir.AluOpType.bypass,
                        replica_groups=[list(range(NCORES))],
                        ins=[cc_in[0:RA, :]],
                        outs=[cc_fa[:, :]],
                    )
            nc.gpsimd.collective_compute(
                "AllGather", mybir.AluOpType.bypass,
                replica_groups=[list(range(NCORES))],
                ins=[cc_in[RA:NT * P, :]],
                outs=[cc_fb[:, :]],
            )

            # ---- phase B: aggregate per dst tile ----
            qn = 0
            for t in range(NT):
                cn = chunks[t][0] + chunks[t][1]
                g_t = ggp.tile([P, cn * P], f16, tag="g")
                oh_t = ohp.tile([P, cn * P], f16, tag="oh")
                goff = 0
                for h in range(2):
                    ch = chunks[t][h]
                    # <=1024-idx sub-gathers: single_packet coalesces each
                    # into one 64-desc packet per engine, which the SDMA
                    # drains pipelined (~4x the per-descriptor rate of
                    # descriptor-per-packet mode; >64-desc packets hang).
                    coff = 0
                    while coff < ch:
                        sub = min(8, ch - coff)
                        K = sub * P
                        qo = q0[t][h] + coff * 8
                        nc.gpsimd.dma_gather(
                            out_ap=g_t[:, (goff + coff) * P:
                                       (goff + coff + sub) * P]
                                .rearrange("p (j f) -> p j f", f=N_HID),
                            in_ap=(cc_fa if h == 0 else cc_fb)[:, :],
                            idxs_ap=gidx_sb[:, qo:qo + K // 16],
                            num_idxs=K,
                            num_idxs_reg=K,
                            elem_size=N_HID,
                            single_packet=True,
                            queue_num=qn % 4,
                        )
                        qn += 1
                        coff += sub
                    goff += ch
                # one-hot: oh[p, (c n)] = (iota[n] == dstrel[p, c])
                nc.vector.tensor_tensor(
                    out=oh_t[:].rearrange("p (c n) -> p c n", n=P),
                    in0=iota_sb[:].rearrange("p (one n) -> p one n", one=1)
                        .to_broadcast([P, cn, P]),
                    in1=dstr_sb[:, ct0[t][0]:ct0[t][0] + cn]
                        .rearrange("p (c one) -> p c one", one=1)
                        .to_broadcast([P, cn, P]),
                    op=mybir.AluOpType.is_equal,
                )
                pa = psa.tile([P, N_HID], f32, tag="pa")
                for j in range(cn):
                    nc.tensor.matmul(pa[:],
                                     lhsT=oh_t[:, j * P:(j + 1) * P],
                                     rhs=g_t[:, j * P:(j + 1) * P],
                                     start=(j == 0), stop=False)
                # self loops: += I^T @ H'_local  (H' rows for this tile)
                nc.tensor.matmul(pa[:], lhsT=idn_sb[:],
                                 rhs=hp_all[:, t * P:(t + 1) * P],
                                 start=False, stop=True)
                # epilogue on ACT + PE only (DVE stalls badly while the Q7
                # gather ucode is hammering SBUF): scale rows by dinv[dst],
                # transpose to [hid, node], then relu(x + b1) with b1 as the
                # per-partition ACT bias in the transposed layout.
                hr = hrp.tile([P, N_HID], f32, tag="hr")
                nc.scalar.activation(hr[:], pa[:],
                                     mybir.ActivationFunctionType.Copy,
                                     scale=dinv_sb[:, t:t + 1])
                ptr = pst.tile([P, P], f32, tag="pt")
                nc.tensor.transpose(out=ptr[:], in_=hr[:], identity=ident[:])
                nc.scalar.activation(hT_sb[:, t * P:(t + 1) * P], ptr[:],
                                     mybir.ActivationFunctionType.Relu,
                                     bias=b1_sb[:, 0:1])

            # ---- phase C: segment max pooling + linear head + log_softmax ----
            pooled = constp.tile([P, GPC], f32)
            for k in range(GPC):
                nc.vector.reduce_max(pooled[:, k:k + 1],
                                     hT_sb[:, gb_local[k]:gb_local[k + 1]],
                                     axis=mybir.AxisListType.X)
            pl = psa.tile([GPC, N_CLASSES], f32, tag="pl")
            nc.tensor.matmul(pl[:], lhsT=pooled[:], rhs=w2_sb[:],
                             start=True, stop=True)
            ls = constp.tile([GPC, N_CLASSES], f32)
            nc.vector.tensor_add(out=ls[:], in0=pl[:], in1=b2_sb[:])
            mx = constp.tile([GPC, 1], f32)
            nc.vector.reduce_max(mx[:], ls[:], axis=mybir.AxisListType.X)
            xm = constp.tile([GPC, N_CLASSES], f32)
            nc.vector.tensor_scalar_sub(xm[:], ls[:], mx[:, 0:1])
            ex = constp.tile([GPC, N_CLASSES], f32)
            nc.scalar.activation(ex[:], xm[:],
                                 mybir.ActivationFunctionType.Exp)
            sm = constp.tile([GPC, 1], f32)
            nc.vector.reduce_sum(sm[:], ex[:], axis=mybir.AxisListType.X)
            lg = constp.tile([GPC, 1], f32)
            nc.scalar.activation(lg[:], sm[:],
                                 mybir.ActivationFunctionType.Ln)
            of = constp.tile([GPC, N_CLASSES], f32)
            nc.vector.tensor_scalar_sub(of[:], xm[:], lg[:, 0:1])
            nc.sync.dma_start(out=out.ap()[:, :], in_=of[:])

    nc.compile()
    return nc


def _install_ntff_hook():
    """Provide antenv.axon_hooks.get_axon_ntff_profile_hook when the agent
    image lacks it (ctypes bridge into libaxon_pjrt.so, mirroring
    trn_boot._ntff_profile_via_ctypes)."""
    import contextlib
    import ctypes
    import sys
    import types
    try:
        import antenv.axon_hooks  # noqa: F401
        return
    except ImportError:
        pass
    the_hook = None
    try:
        lib = ctypes.CDLL("/opt/axon/libaxon_pjrt.so")
        if hasattr(lib, "axon_start_nrt_profile"):
            lib.axon_start_nrt_profile.argtypes = [
                ctypes.POINTER(ctypes.c_int64), ctypes.c_size_t]
            lib.axon_start_nrt_profile.restype = ctypes.c_int64
            lib.axon_stop_nrt_profile.argtypes = [ctypes.c_char_p]
            lib.axon_stop_nrt_profile.restype = ctypes.c_int64

            @contextlib.contextmanager
            def _hook(output_dir, device_ids):
                import jax
                jax.devices()
                if device_ids:
                    ids = (ctypes.c_int64 * len(device_ids))(*device_ids)
                    rc = lib.axon_start_nrt_profile(ids, len(device_ids))
                else:
                    rc = lib.axon_start_nrt_profile(None, 0)
                if rc != 0:
                    raise RuntimeError(f"axon_start_nrt_profile rc={rc}")
                try:
                    yield
                finally:
                    n = lib.axon_stop_nrt_profile(str(output_dir).encode())
                    print(f"ntff profile: {n} file(s) -> {output_dir}")

            the_hook = _hook
    except OSError:
        pass
    mod = types.ModuleType("antenv.axon_hooks")
    mod.get_axon_ntff_profile_hook = lambda: the_hook
    mod.set_axon_ntff_profile_hook = lambda h: None
    import antenv
    antenv.axon_hooks = mod
    sys.modules["antenv.axon_hooks"] = mod


def kernel(x, W1, b1, W2, b2, edge_index, batch, _trace=False, _trace_kwargs=None):
    global LAST_RESULTS
    import shutil
    from concourse import bass_utils
    if _trace:
        _install_ntff_hook()
        # no fish/S3 in this container; keep artifacts local
        bass_utils.upload_artifacts = lambda tmpdir: tmpdir
        shutil.rmtree("/tmp/gnn_neff", ignore_errors=True)

    chunks, gb_local, in_maps = _host_prep(x, W1, b1, W2, b2,
                                           edge_index, batch)
    key = (chunks, gb_local)
    nc = _PROGRAM_CACHE.get(key)
    if nc is None:
        nc = _build_program(chunks, gb_local)
        _PROGRAM_CACHE[key] = nc

    res = bass_utils.run_bass_kernel_spmd(
        nc, in_maps, core_ids=list(range(NCORES)),
        trace=_trace, tmpdir="/tmp/gnn_neff" if _trace else None,
        **(_trace_kwargs or {}))
    LAST_RESULTS = res
    return np.concatenate([res.results[c]["out"] for c in range(NCORES)],
                          axis=0)
